# revision 2
# baseline (speedup 1.0000x reference)
"""GAT-with-edge-attr Trainium kernel v2: AllGather-sharded table + For_i loops.

Edges sorted by dst, sharded as contiguous 128-node blocks across 8 cores ->
segment softmax/aggregation is core-local. Each core projects only its own
node shard (h | a_src | a_dst | x packed per row, one fused matmul including a
PE identity-transpose for the x columns), then a single AllGather assembles
the full fp16 node table on every core. Per 128-edge chunk a one-hot
(edge x node) matrix turns gather-scatter into PE matmuls; h/a_src rows are
fetched by src via indirect DMA from the gathered table. Hardware For_i loops
keep the program ~150 instructions so NEFF compile/load stays sub-second.
"""
import sys
sys.path.insert(0, '/opt/trn_rl_repo')
import numpy as np
import concourse.bass as bass
import concourse.mybir as mybir
from concourse.bass import ts
from concourse.tile import TileContext
from concourse import bacc

f32, f16, i32 = mybir.dt.float32, mybir.dt.float16, mybir.dt.int32
AF = mybir.ActivationFunctionType
OP = mybir.AluOpType

P = 128
D = 128
H = 2
CC = 128          # channels per head
ROW = 392         # table row: h0|h1(256) | asrc(2) | adst(2) | x(128) | pad(4)
TCOL = 388        # written table cols (pad never read)
SEG = H * CC + 2  # 258: rhs segment (scaled h0 | scaled h1 | ex pair)
SEGP = 512        # rhs segment stride, 1KB-aligned: unaligned matmul-rhs SBUF
                  # offsets trigger a pathological (~60s) terminal load path
LEAKY = 0.2
SM_EPS = 1e-16
LN_EPS = 1e-5
NCORES = 8
# Wall column layout (f32): W_lin 0:256 | p_src 256:258 | p_dst 258:260 |
# identity 260:388 | uaug[0:9 rows] 388:390 | iota 390:518 | bias_bcast 518:646
WCOLS = 646


def build_kernel(NB, NCH):
    """NB: node blocks per core; NCH: 128-edge chunks per block."""
    SLOTS = NB * NCH * P
    ECH = NCH * P
    NSH = NB * P                      # nodes per shard
    NPP = NSH * NCORES                # total padded nodes
    nc = bacc.Bacc("TRN2", target_bir_lowering=False, num_swdge_queues=4,
                   num_devices=NCORES)

    # ---- inputs ----
    Wall = nc.dram_tensor("Wall", [P, WCOLS], f32, kind="ExternalInput")
    xTs = nc.dram_tensor("xTs", [P, NSH], f32, kind="ExternalInput")
    eaT = nc.dram_tensor("eaT", [9, SLOTS], f16, kind="ExternalInput")
    srcidx = nc.dram_tensor("srcidx", [P, NB * NCH], i32, kind="ExternalInput")
    dstln = nc.dram_tensor("dstln", [P, NB * NCH], f16, kind="ExternalInput")
    out = nc.dram_tensor("out", [NSH, P], f16, kind="ExternalOutput")
    # ---- internal ----
    Tsh = nc.dram_tensor("Tsh", [NSH, ROW], f16)
    T = nc.dram_tensor("T", [NPP, ROW], f16, addr_space="Shared")

    with TileContext(nc) as tc:
        with tc.tile_pool(name="const", bufs=1) as cpool:
            Wall_sb = cpool.tile([P, WCOLS], f32)
            nc.sync.dma_start(out=Wall_sb[:], in_=Wall[:, :])
            uaug_sb = cpool.tile([16, 2], f16)
            nc.vector.tensor_copy(out=uaug_sb[0:9, :], in_=Wall_sb[0:9, 388:390])
            iota_sb = cpool.tile([P, P], f16)
            nc.vector.tensor_copy(out=iota_sb[:], in_=Wall_sb[:, 390:518])
            bias_sb = cpool.tile([P, P], f16)
            nc.vector.tensor_copy(out=bias_sb[:], in_=Wall_sb[:, 518:646])

            # ================= P1: own-shard table build =================
            with tc.tile_pool(name="p1", bufs=3) as p1, \
                 tc.tile_pool(name="p1ps", bufs=2, space="PSUM") as p1ps:
                with tc.For_i(0, NB, 1) as j:
                    xt = p1.tile([P, P], f32, tag="xt")
                    nc.sync.dma_start(out=xt[:], in_=xTs[:, ts(j, P)])
                    ps = p1ps.tile([P, TCOL], f32, tag="ps")
                    nc.tensor.matmul(out=ps[:], lhsT=xt[:], rhs=Wall_sb[:, 0:TCOL],
                                     start=True, stop=True)
                    tt = p1.tile([P, TCOL], f16, tag="tt")
                    nc.vector.tensor_copy(out=tt[:, 0:194], in_=ps[:, 0:194])
                    nc.scalar.activation(out=tt[:, 194:TCOL], in_=ps[:, 194:TCOL],
                                         func=AF.Copy)
                    nc.sync.dma_start(out=Tsh[ts(j, P), 0:TCOL], in_=tt[:])

            tc.strict_bb_all_engine_barrier()
            nc.gpsimd.collective_compute(
                "AllGather", OP.bypass,
                replica_groups=[list(range(NCORES))],
                ins=[Tsh[:, :].opt()],
                outs=[T[:, :].opt()],
            )
            tc.strict_bb_all_engine_barrier()

            # ================= P2: edge blocks =================
            with tc.tile_pool(name="p2", bufs=2) as p2, \
                 tc.tile_pool(name="p2b", bufs=2) as p2b, \
                 tc.tile_pool(name="agg", bufs=2, space="PSUM") as aggps, \
                 tc.tile_pool(name="sps", bufs=2, space="PSUM") as sps:
                with tc.For_i(0, NB, 1) as b:
                    # ---- block loads ----
                    eat = p2.tile([9, ECH], f16, tag="eat")
                    nc.sync.dma_start(out=eat[:], in_=eaT[:, ts(b, ECH)])
                    dl = p2.tile([P, NCH], f16, tag="dl")
                    nc.sync.dma_start(out=dl[:], in_=dstln[:, ts(b, NCH)])
                    its = p2.tile([P, NCH], i32, tag="its")
                    nc.sync.dma_start(out=its[:], in_=srcidx[:, ts(b, NCH)])
                    ad16 = p2b.tile([P, 2], f16, tag="ad16")
                    nc.sync.dma_start(out=ad16[:], in_=Tsh[ts(b, P), 256 + 2:256 + 4])
                    xres = p2b.tile([P, P], f16, tag="xres")
                    nc.sync.dma_start(out=xres[:], in_=Tsh[ts(b, P), 260:388])

                    # ---- gather table rows by src ----
                    gt = p2.tile([P, NCH * ROW], f16, tag="gt")
                    for g in range(NCH):
                        nc.gpsimd.indirect_dma_start(
                            out=gt[:, g * ROW:(g + 1) * ROW], out_offset=None,
                            in_=T[:, :],
                            in_offset=bass.IndirectOffsetOnAxis(ap=its[:, g:g + 1], axis=0))

                    # ---- one-hot + transposed one-hot ----
                    oh = p2.tile([P, ECH], f16, tag="oh")
                    nc.vector.tensor_tensor(
                        out=oh[:].rearrange("p (k f) -> p k f", k=NCH),
                        in0=dl[:].rearrange("p (k o) -> p k o", o=1).to_broadcast([P, NCH, P]),
                        in1=iota_sb[:].rearrange("p (o f) -> p o f", o=1).to_broadcast([P, NCH, P]),
                        op=OP.is_equal)
                    ohT = p2.tile([P, ECH], f16, tag="ohT")
                    nc.sync.dma_start_transpose(
                        out=ohT[:].rearrange("p (k f) -> p k f", k=NCH), in_=oh[:])

                    # ---- scores: aedge + adst into PSUM ----
                    sp = sps.tile([P, 2 * NCH], f32, tag="sp", space="PSUM")
                    for k in range(NCH):
                        nc.tensor.matmul(out=sp[:, 2 * k:2 * k + 2],
                                         lhsT=eat[:, k * P:(k + 1) * P],
                                         rhs=uaug_sb[0:9, :], start=True, stop=False)
                        nc.tensor.matmul(out=sp[:, 2 * k:2 * k + 2],
                                         lhsT=ohT[:, k * P:(k + 1) * P],
                                         rhs=ad16[:], start=False, stop=True)
                    # s = sp + asrc(gathered)
                    s32 = p2b.tile([P, 2 * NCH], f32, tag="s32")
                    nc.vector.tensor_tensor(
                        out=s32[:].rearrange("p (k f) -> p k f", k=NCH),
                        in0=sp[:].rearrange("p (k f) -> p k f", k=NCH),
                        in1=gt[:].rearrange("p (k f) -> p k f", k=NCH)[:, :, 256:258],
                        op=OP.add)
                    # ex = exp(leaky(s)) = max(exp(s), exp(0.2 s))
                    e1 = p2b.tile([P, 2 * NCH], f32, tag="e1")
                    nc.scalar.activation(out=e1[:], in_=s32[:], func=AF.Exp)
                    e2 = p2b.tile([P, 2 * NCH], f32, tag="e2")
                    nc.scalar.activation(out=e2[:], in_=s32[:], func=AF.Exp, scale=LEAKY)
                    ex32 = p2b.tile([P, 2 * NCH], f32, tag="ex32")
                    nc.vector.tensor_tensor(out=ex32[:], in0=e1[:], in1=e2[:], op=OP.max)
                    ex16 = p2b.tile([P, 2 * NCH], f16, tag="ex16")
                    nc.vector.tensor_copy(out=ex16[:], in_=ex32[:])

                    # ---- scaled rhs: [scaled_h0 | scaled_h1 | ex pair] per chunk ----
                    rhs = p2.tile([P, NCH * SEGP], f16, tag="rhs")
                    for k in range(NCH):
                        nc.vector.tensor_scalar_mul(
                            out=rhs[:, k * SEGP:k * SEGP + CC],
                            in0=gt[:, k * ROW:k * ROW + CC],
                            scalar1=ex32[:, 2 * k:2 * k + 1])
                        nc.scalar.activation(
                            out=rhs[:, k * SEGP + CC:k * SEGP + 2 * CC],
                            in_=gt[:, k * ROW + CC:k * ROW + 2 * CC],
                            func=AF.Copy, scale=ex32[:, 2 * k + 1:2 * k + 2])
                    nc.vector.tensor_copy(
                        out=rhs[:].rearrange("p (k f) -> p k f", k=NCH)[:, :, 256:258],
                        in_=ex16[:].rearrange("p (k f) -> p k f", k=NCH))

                    # ---- scatter-accumulate: one matmul per chunk ----
                    aggp = aggps.tile([P, SEG], f32, tag="aggp", space="PSUM")
                    for k in range(NCH):
                        nc.tensor.matmul(out=aggp[:], lhsT=oh[:, k * P:(k + 1) * P],
                                         rhs=rhs[:, k * SEGP:k * SEGP + SEG],
                                         start=(k == 0), stop=(k == NCH - 1))

                    # ---- epilogue: normalize, head-mean, +bias, residual, LN ----
                    dn = p2b.tile([P, 2], f32, tag="dn")
                    nc.vector.tensor_scalar_add(out=dn[:], in0=aggp[:, 256:258], scalar1=SM_EPS)
                    rr = p2b.tile([P, 2], f32, tag="rr")
                    nc.vector.reciprocal(out=rr[:], in_=dn[:])
                    nc.vector.tensor_scalar_mul(out=rr[:], in0=rr[:], scalar1=0.5)
                    t0 = p2b.tile([P, P], f32, tag="t0")
                    nc.vector.tensor_scalar_mul(out=t0[:], in0=aggp[:, 0:CC], scalar1=rr[:, 0:1])
                    t1 = p2b.tile([P, P], f32, tag="t1")
                    nc.vector.tensor_scalar_mul(out=t1[:], in0=aggp[:, CC:2 * CC], scalar1=rr[:, 1:2])
                    y = p2b.tile([P, P], f32, tag="y")
                    nc.vector.tensor_tensor(out=y[:], in0=t0[:], in1=t1[:], op=OP.add)
                    nc.vector.tensor_tensor(out=y[:], in0=y[:], in1=bias_sb[:], op=OP.add)
                    nc.vector.tensor_tensor(out=y[:], in0=y[:], in1=xres[:], op=OP.add)
                    # mean / var / normalize
                    mu = p2b.tile([P, 1], f32, tag="mu")
                    nc.vector.tensor_reduce(out=mu[:], in_=y[:], axis=mybir.AxisListType.X, op=OP.add)
                    nc.vector.tensor_scalar_mul(out=mu[:], in0=mu[:], scalar1=1.0 / P)
                    ymu = p2b.tile([P, P], f32, tag="ymu")
                    nc.vector.tensor_scalar_sub(out=ymu[:], in0=y[:], scalar1=mu[:, 0:1])
                    scr = p2b.tile([P, P], f32, tag="scr")
                    vs = p2b.tile([P, 1], f32, tag="vs")
                    nc.scalar.activation(out=scr[:], in_=ymu[:], func=AF.Square, accum_out=vs[:])
                    vsn = p2b.tile([P, 1], f32, tag="vsn")
                    nc.vector.tensor_scalar(out=vsn[:], in0=vs[:], scalar1=1.0 / P,
                                            scalar2=LN_EPS, op0=OP.mult, op1=OP.add)
                    sd = p2b.tile([P, 1], f32, tag="sd")
                    nc.scalar.activation(out=sd[:], in_=vsn[:], func=AF.Sqrt)
                    rs = p2b.tile([P, 1], f32, tag="rs")
                    nc.vector.reciprocal(out=rs[:], in_=sd[:])
                    ob = p2b.tile([P, P], f16, tag="ob")
                    nc.vector.tensor_scalar_mul(out=ob[:], in0=ymu[:], scalar1=rs[:, 0:1])
                    nc.sync.dma_start(out=out[ts(b, P), :], in_=ob[:])

    nc.compile()
    return nc


def prep_inputs(x, edge_index, edge_attr, W_ep, b_ep, W_lin, att_src, att_dst,
                W_le, att_edge, bias_gat, ln_gamma, ln_beta, ncores=NCORES):
    """Host-side layout/index prep. Returns (in_maps, meta)."""
    N = x.shape[0]
    ED = edge_attr.shape[1]
    nblk_tot = (N + P - 1) // P
    NB = (nblk_tot + ncores - 1) // ncores
    NSH = NB * P
    NPP = NSH * ncores

    x = np.asarray(x, np.float32)
    edge_attr = np.asarray(edge_attr, np.float32)
    # param folding (fp64 for exactness)
    W_le_h = np.asarray(W_le, np.float64).reshape(D, H, CC)
    v = np.einsum('dhc,hc->dh', W_le_h, np.asarray(att_edge, np.float64))
    u = np.asarray(W_ep, np.float64) @ v          # [ED, H]
    c0 = np.asarray(b_ep, np.float64) @ v         # [H]
    W_lin_h = np.asarray(W_lin, np.float64).reshape(D, H, CC)
    p_src = np.einsum('dhc,hc->dh', W_lin_h, np.asarray(att_src, np.float64))
    p_dst = np.einsum('dhc,hc->dh', W_lin_h, np.asarray(att_dst, np.float64))

    Wall = np.zeros((P, WCOLS), np.float32)
    Wall[:, 0:256] = np.asarray(W_lin, np.float32)
    Wall[:, 256:258] = p_src.astype(np.float32)
    Wall[:, 258:260] = p_dst.astype(np.float32)
    Wall[:, 260:388] = np.eye(P, dtype=np.float32)
    Wall[0:ED, 388:390] = u.astype(np.float32)
    Wall[ED, 388:390] = c0.astype(np.float32)     # ones row of eaT picks this up
    Wall[:, 390:518] = np.tile(np.arange(P, dtype=np.float32), (P, 1))
    Wall[:, 518:646] = np.tile(np.asarray(bias_gat, np.float32), (P, 1))

    xpadT = np.zeros((P, NPP), np.float32)
    xpadT[:, 0:N] = x.T

    # edge sort + per-core slotting
    src = np.asarray(edge_index[0], np.int64)
    dst = np.asarray(edge_index[1], np.int64)
    order = np.argsort(dst, kind='stable')
    src_s, dst_s = src[order], dst[order]
    ea_s = edge_attr[order]
    blk = (dst_s // P).astype(np.int64)
    counts = np.bincount(blk, minlength=NB * ncores)
    NCH = int(np.max((counts + P - 1) // P))
    SLOTS = NB * NCH * P

    bstart = np.zeros(NB * ncores + 1, np.int64)
    np.cumsum(counts, out=bstart[1:])

    in_maps = []
    for c in range(ncores):
        srcidx = np.zeros(SLOTS, np.int32)
        dstln = np.full(SLOTS, -1.0, np.float16)
        eaT = np.zeros((9, SLOTS), np.float16)
        for b in range(NB):
            g = c * NB + b
            k = counts[g]
            sl = slice(bstart[g], bstart[g] + k)
            o = b * NCH * P
            srcidx[o:o + k] = src_s[sl]
            dstln[o:o + k] = (dst_s[sl] - g * P).astype(np.float16)
            eaT[0:ED, o:o + k] = ea_s[sl].T.astype(np.float16)
            eaT[ED, o:o + k] = 1.0
        in_maps.append({
            "Wall": Wall,
            "xTs": xpadT[:, c * NSH:(c + 1) * NSH].copy(),
            "eaT": eaT,
            "srcidx": srcidx.reshape(NB * NCH, P).T.copy(),
            "dstln": dstln.reshape(NB * NCH, P).T.copy(),
        })
    meta = dict(NB=NB, NCH=NCH, N=N, ncores=ncores,
                gamma=np.asarray(ln_gamma, np.float32),
                beta=np.asarray(ln_beta, np.float32))
    return in_maps, meta


def assemble_output(results, meta):
    outs = [r["out"] for r in results]
    full = np.concatenate(outs, axis=0).astype(np.float32)
    full = full * meta["gamma"] + meta["beta"]
    return full[:meta["N"]]


def kernel(**inputs):
    """Full-input GAT kernel: shards edges by dst across 8 NeuronCores."""
    from concourse import bass_utils
    inputs = {k: np.asarray(v) for k, v in inputs.items()}
    in_maps, meta = prep_inputs(**inputs)
    nc = build_kernel(meta["NB"], meta["NCH"])
    res = bass_utils.run_bass_kernel_spmd(nc, in_maps, core_ids=list(range(meta["ncores"])))
    return assemble_output(res.results, meta).astype(np.float32)


# revision 3
# speedup vs baseline: 1.2657x; 1.2657x over previous
"""GAT-with-edge-attr Trainium kernel v2: AllGather-sharded table + For_i loops.

Edges sorted by dst, sharded as contiguous 128-node blocks across 8 cores ->
segment softmax/aggregation is core-local. Each core projects only its own
node shard (h | a_src | a_dst | x packed per row, one fused matmul including a
PE identity-transpose for the x columns), then a single AllGather assembles
the full fp16 node table on every core. Per 128-edge chunk a one-hot
(edge x node) matrix turns gather-scatter into PE matmuls; h/a_src rows are
fetched by src via indirect DMA from the gathered table. Hardware For_i loops
keep the program ~150 instructions so NEFF compile/load stays sub-second.
"""
import sys
sys.path.insert(0, '/opt/trn_rl_repo')
import numpy as np
import concourse.bass as bass
import concourse.mybir as mybir
from concourse.bass import ts
from concourse.tile import TileContext
from concourse import bacc

f32, f16, i32 = mybir.dt.float32, mybir.dt.float16, mybir.dt.int32
AF = mybir.ActivationFunctionType
OP = mybir.AluOpType

P = 128
D = 128
H = 2
CC = 128          # channels per head
ROW = 392         # table row: h0|h1(256) | asrc(2) | adst(2) | x(128) | pad(4)
TCOL = 388        # written table cols (pad never read)
SEG = H * CC + 2  # 258: rhs segment (scaled h0 | scaled h1 | ex pair)
SEGP = 512        # rhs segment stride, 1KB-aligned: unaligned matmul-rhs SBUF
                  # offsets trigger a pathological (~60s) terminal load path
LEAKY = 0.2
SM_EPS = 1e-16
LN_EPS = 1e-5
NCORES = 8
# Wall column layout (f32): W_lin 0:256 | p_src 256:258 | p_dst 258:260 |
# identity 260:388 | uaug[0:9 rows] 388:390 | iota 390:518 | bias_bcast 518:646
WCOLS = 646


def build_kernel(NB, NCH):
    """NB: node blocks per core; NCH: 128-edge chunks per block."""
    SLOTS = NB * NCH * P
    ECH = NCH * P
    NSH = NB * P                      # nodes per shard
    NPP = NSH * NCORES                # total padded nodes
    nc = bacc.Bacc("TRN2", target_bir_lowering=False, num_swdge_queues=4,
                   num_devices=NCORES)

    # ---- inputs ----
    Wall = nc.dram_tensor("Wall", [P, WCOLS], f16, kind="ExternalInput")
    xTs = nc.dram_tensor("xTs", [P, NSH], f16, kind="ExternalInput")
    eaT = nc.dram_tensor("eaT", [9, SLOTS], f16, kind="ExternalInput")
    srcidx = nc.dram_tensor("srcidx", [P, NB * NCH], i32, kind="ExternalInput")
    dstln = nc.dram_tensor("dstln", [P, NB * NCH], f16, kind="ExternalInput")
    out = nc.dram_tensor("out", [NSH, P], f16, kind="ExternalOutput")
    # ---- internal ----
    Tsh = nc.dram_tensor("Tsh", [NSH, ROW], f16)
    T = nc.dram_tensor("T", [NPP, ROW], f16, addr_space="Shared")

    with TileContext(nc) as tc:
        with tc.tile_pool(name="const", bufs=1) as cpool:
            Wall_sb = cpool.tile([P, WCOLS], f16)
            nc.sync.dma_start(out=Wall_sb[:], in_=Wall[:, :])
            uaug_sb = Wall_sb[0:9, 388:390]
            iota_sb = Wall_sb[:, 390:518]
            bias_sb = Wall_sb[:, 518:646]

            # ================= P1: own-shard table build =================
            with tc.tile_pool(name="p1", bufs=3) as p1, \
                 tc.tile_pool(name="p1ps", bufs=2, space="PSUM") as p1ps:
                with tc.For_i(0, NB, 1) as j:
                    xt = p1.tile([P, P], f16, tag="xt")
                    nc.sync.dma_start(out=xt[:], in_=xTs[:, ts(j, P)])
                    ps = p1ps.tile([P, TCOL], f32, tag="ps")
                    nc.tensor.matmul(out=ps[:], lhsT=xt[:], rhs=Wall_sb[:, 0:TCOL],
                                     start=True, stop=True)
                    tt = p1.tile([P, TCOL], f16, tag="tt")
                    nc.vector.tensor_copy(out=tt[:, 0:194], in_=ps[:, 0:194])
                    nc.scalar.activation(out=tt[:, 194:TCOL], in_=ps[:, 194:TCOL],
                                         func=AF.Copy)
                    nc.sync.dma_start(out=Tsh[ts(j, P), 0:TCOL], in_=tt[:])

            tc.strict_bb_all_engine_barrier()
            nc.gpsimd.collective_compute(
                "AllGather", OP.bypass,
                replica_groups=[list(range(NCORES))],
                ins=[Tsh[:, :].opt()],
                outs=[T[:, :].opt()],
            )
            tc.strict_bb_all_engine_barrier()

            # ================= P2: edge blocks =================
            with tc.tile_pool(name="p2", bufs=2) as p2, \
                 tc.tile_pool(name="p2b", bufs=2) as p2b, \
                 tc.tile_pool(name="agg", bufs=2, space="PSUM") as aggps, \
                 tc.tile_pool(name="sps", bufs=2, space="PSUM") as sps:
                with tc.For_i(0, NB, 1) as b:
                    # ---- block loads ----
                    eat = p2.tile([9, ECH], f16, tag="eat")
                    nc.sync.dma_start(out=eat[:], in_=eaT[:, ts(b, ECH)])
                    dl = p2.tile([P, NCH], f16, tag="dl")
                    nc.sync.dma_start(out=dl[:], in_=dstln[:, ts(b, NCH)])
                    its = p2.tile([P, NCH], i32, tag="its")
                    nc.sync.dma_start(out=its[:], in_=srcidx[:, ts(b, NCH)])
                    ad16 = p2b.tile([P, 2], f16, tag="ad16")
                    nc.sync.dma_start(out=ad16[:], in_=Tsh[ts(b, P), 256 + 2:256 + 4])
                    xres = p2b.tile([P, P], f16, tag="xres")
                    nc.sync.dma_start(out=xres[:], in_=Tsh[ts(b, P), 260:388])

                    # ---- gather table rows by src ----
                    gt = p2.tile([P, NCH * ROW], f16, tag="gt")
                    for g in range(NCH):
                        nc.gpsimd.indirect_dma_start(
                            out=gt[:, g * ROW:(g + 1) * ROW], out_offset=None,
                            in_=T[:, :],
                            in_offset=bass.IndirectOffsetOnAxis(ap=its[:, g:g + 1], axis=0))

                    # ---- one-hot + transposed one-hot ----
                    oh = p2.tile([P, ECH], f16, tag="oh")
                    nc.vector.tensor_tensor(
                        out=oh[:].rearrange("p (k f) -> p k f", k=NCH),
                        in0=dl[:].rearrange("p (k o) -> p k o", o=1).to_broadcast([P, NCH, P]),
                        in1=iota_sb.rearrange("p (o f) -> p o f", o=1).to_broadcast([P, NCH, P]),
                        op=OP.is_equal)
                    ohT = p2.tile([P, ECH], f16, tag="ohT")
                    nc.sync.dma_start_transpose(
                        out=ohT[:].rearrange("p (k f) -> p k f", k=NCH), in_=oh[:])

                    # ---- scores: aedge + adst into PSUM ----
                    sp = sps.tile([P, 2 * NCH], f32, tag="sp", space="PSUM")
                    for k in range(NCH):
                        nc.tensor.matmul(out=sp[:, 2 * k:2 * k + 2],
                                         lhsT=eat[:, k * P:(k + 1) * P],
                                         rhs=uaug_sb, start=True, stop=False)
                        nc.tensor.matmul(out=sp[:, 2 * k:2 * k + 2],
                                         lhsT=ohT[:, k * P:(k + 1) * P],
                                         rhs=ad16[:], start=False, stop=True)
                    # s = sp + asrc(gathered)
                    s32 = p2b.tile([P, 2 * NCH], f32, tag="s32")
                    nc.vector.tensor_tensor(
                        out=s32[:].rearrange("p (k f) -> p k f", k=NCH),
                        in0=sp[:].rearrange("p (k f) -> p k f", k=NCH),
                        in1=gt[:].rearrange("p (k f) -> p k f", k=NCH)[:, :, 256:258],
                        op=OP.add)
                    # ex = exp(leaky(s)) = max(exp(s), exp(0.2 s))
                    e1 = p2b.tile([P, 2 * NCH], f32, tag="e1")
                    nc.scalar.activation(out=e1[:], in_=s32[:], func=AF.Exp)
                    e2 = p2b.tile([P, 2 * NCH], f32, tag="e2")
                    nc.scalar.activation(out=e2[:], in_=s32[:], func=AF.Exp, scale=LEAKY)
                    ex32 = p2b.tile([P, 2 * NCH], f32, tag="ex32")
                    nc.vector.tensor_tensor(out=ex32[:], in0=e1[:], in1=e2[:], op=OP.max)
                    ex16 = p2b.tile([P, 2 * NCH], f16, tag="ex16")
                    nc.vector.tensor_copy(out=ex16[:], in_=ex32[:])

                    # ---- scaled rhs: [scaled_h0 | scaled_h1 | ex pair] per chunk ----
                    rhs = p2.tile([P, NCH * SEGP], f16, tag="rhs")
                    for k in range(NCH):
                        nc.vector.tensor_scalar_mul(
                            out=rhs[:, k * SEGP:k * SEGP + CC],
                            in0=gt[:, k * ROW:k * ROW + CC],
                            scalar1=ex32[:, 2 * k:2 * k + 1])
                        nc.scalar.activation(
                            out=rhs[:, k * SEGP + CC:k * SEGP + 2 * CC],
                            in_=gt[:, k * ROW + CC:k * ROW + 2 * CC],
                            func=AF.Copy, scale=ex32[:, 2 * k + 1:2 * k + 2])
                    nc.vector.tensor_copy(
                        out=rhs[:].rearrange("p (k f) -> p k f", k=NCH)[:, :, 256:258],
                        in_=ex16[:].rearrange("p (k f) -> p k f", k=NCH))

                    # ---- scatter-accumulate: one matmul per chunk ----
                    aggp = aggps.tile([P, SEG], f32, tag="aggp", space="PSUM")
                    for k in range(NCH):
                        nc.tensor.matmul(out=aggp[:], lhsT=oh[:, k * P:(k + 1) * P],
                                         rhs=rhs[:, k * SEGP:k * SEGP + SEG],
                                         start=(k == 0), stop=(k == NCH - 1))

                    # ---- epilogue: normalize, head-mean, +bias, residual, LN ----
                    dn = p2b.tile([P, 2], f32, tag="dn")
                    nc.vector.tensor_scalar_add(out=dn[:], in0=aggp[:, 256:258], scalar1=SM_EPS)
                    rr = p2b.tile([P, 2], f32, tag="rr")
                    nc.vector.reciprocal(out=rr[:], in_=dn[:])
                    nc.vector.tensor_scalar_mul(out=rr[:], in0=rr[:], scalar1=0.5)
                    t0 = p2b.tile([P, P], f32, tag="t0")
                    nc.vector.tensor_scalar_mul(out=t0[:], in0=aggp[:, 0:CC], scalar1=rr[:, 0:1])
                    t1 = p2b.tile([P, P], f32, tag="t1")
                    nc.vector.tensor_scalar_mul(out=t1[:], in0=aggp[:, CC:2 * CC], scalar1=rr[:, 1:2])
                    y = p2b.tile([P, P], f32, tag="y")
                    nc.vector.tensor_tensor(out=y[:], in0=t0[:], in1=t1[:], op=OP.add)
                    nc.vector.tensor_tensor(out=y[:], in0=y[:], in1=bias_sb, op=OP.add)
                    nc.vector.tensor_tensor(out=y[:], in0=y[:], in1=xres[:], op=OP.add)
                    # mean / var / normalize
                    mu = p2b.tile([P, 1], f32, tag="mu")
                    nc.vector.tensor_reduce(out=mu[:], in_=y[:], axis=mybir.AxisListType.X, op=OP.add)
                    nc.vector.tensor_scalar_mul(out=mu[:], in0=mu[:], scalar1=1.0 / P)
                    ymu = p2b.tile([P, P], f32, tag="ymu")
                    nc.vector.tensor_scalar_sub(out=ymu[:], in0=y[:], scalar1=mu[:, 0:1])
                    scr = p2b.tile([P, P], f32, tag="scr")
                    vs = p2b.tile([P, 1], f32, tag="vs")
                    nc.scalar.activation(out=scr[:], in_=ymu[:], func=AF.Square, accum_out=vs[:])
                    vsn = p2b.tile([P, 1], f32, tag="vsn")
                    nc.vector.tensor_scalar(out=vsn[:], in0=vs[:], scalar1=1.0 / P,
                                            scalar2=LN_EPS, op0=OP.mult, op1=OP.add)
                    sd = p2b.tile([P, 1], f32, tag="sd")
                    nc.scalar.activation(out=sd[:], in_=vsn[:], func=AF.Sqrt)
                    rs = p2b.tile([P, 1], f32, tag="rs")
                    nc.vector.reciprocal(out=rs[:], in_=sd[:])
                    ob = p2b.tile([P, P], f16, tag="ob")
                    nc.vector.tensor_scalar_mul(out=ob[:], in0=ymu[:], scalar1=rs[:, 0:1])
                    nc.sync.dma_start(out=out[ts(b, P), :], in_=ob[:])

    nc.compile()
    return nc


def prep_inputs(x, edge_index, edge_attr, W_ep, b_ep, W_lin, att_src, att_dst,
                W_le, att_edge, bias_gat, ln_gamma, ln_beta, ncores=NCORES):
    """Host-side layout/index prep. Returns (in_maps, meta)."""
    N = x.shape[0]
    ED = edge_attr.shape[1]
    nblk_tot = (N + P - 1) // P
    NB = (nblk_tot + ncores - 1) // ncores
    NSH = NB * P
    NPP = NSH * ncores

    x = np.asarray(x, np.float32)
    edge_attr = np.asarray(edge_attr, np.float32)
    # param folding (fp64 for exactness)
    W_le_h = np.asarray(W_le, np.float64).reshape(D, H, CC)
    v = np.einsum('dhc,hc->dh', W_le_h, np.asarray(att_edge, np.float64))
    u = np.asarray(W_ep, np.float64) @ v          # [ED, H]
    c0 = np.asarray(b_ep, np.float64) @ v         # [H]
    W_lin_h = np.asarray(W_lin, np.float64).reshape(D, H, CC)
    p_src = np.einsum('dhc,hc->dh', W_lin_h, np.asarray(att_src, np.float64))
    p_dst = np.einsum('dhc,hc->dh', W_lin_h, np.asarray(att_dst, np.float64))

    Wall = np.zeros((P, WCOLS), np.float16)
    Wall[:, 0:256] = np.asarray(W_lin, np.float16)
    Wall[:, 256:258] = p_src.astype(np.float16)
    Wall[:, 258:260] = p_dst.astype(np.float16)
    Wall[:, 260:388] = np.eye(P, dtype=np.float16)
    Wall[0:ED, 388:390] = u.astype(np.float16)
    Wall[ED, 388:390] = c0.astype(np.float16)     # ones row of eaT picks this up
    Wall[:, 390:518] = np.tile(np.arange(P, dtype=np.float16), (P, 1))
    Wall[:, 518:646] = np.tile(np.asarray(bias_gat, np.float16), (P, 1))

    xpadT = np.zeros((P, NPP), np.float16)
    xpadT[:, 0:N] = x.T.astype(np.float16)

    # edge sort + per-core slotting
    src = np.asarray(edge_index[0], np.int64)
    dst = np.asarray(edge_index[1], np.int64)
    order = np.argsort(dst, kind='stable')
    src_s, dst_s = src[order], dst[order]
    ea_s = edge_attr[order]
    blk = (dst_s // P).astype(np.int64)
    counts = np.bincount(blk, minlength=NB * ncores)
    NCH = int(np.max((counts + P - 1) // P))
    SLOTS = NB * NCH * P

    bstart = np.zeros(NB * ncores + 1, np.int64)
    np.cumsum(counts, out=bstart[1:])

    # vectorized slotting: flat position of each sorted edge across all cores
    rank = np.arange(len(dst_s), dtype=np.int64) - bstart[blk]
    b_local = blk % NB
    pos = (blk // NB) * SLOTS + b_local * (NCH * P) + rank
    srcidx_a = np.zeros(ncores * SLOTS, np.int32)
    dstln_a = np.full(ncores * SLOTS, -1.0, np.float16)
    eaT_a = np.zeros((9, ncores * SLOTS), np.float16)
    srcidx_a[pos] = src_s
    dstln_a[pos] = (dst_s - blk * P).astype(np.float16)
    eaT_a[0:ED, pos] = ea_s.T.astype(np.float16)
    eaT_a[ED, pos] = 1.0

    in_maps = []
    for c in range(ncores):
        sl = slice(c * SLOTS, (c + 1) * SLOTS)
        in_maps.append({
            "Wall": Wall,
            "xTs": xpadT[:, c * NSH:(c + 1) * NSH].copy(),
            "eaT": eaT_a[:, sl].copy(),
            "srcidx": srcidx_a[sl].reshape(NB * NCH, P).T.copy(),
            "dstln": dstln_a[sl].reshape(NB * NCH, P).T.copy(),
        })
    meta = dict(NB=NB, NCH=NCH, N=N, ncores=ncores,
                gamma=np.asarray(ln_gamma, np.float32),
                beta=np.asarray(ln_beta, np.float32))
    return in_maps, meta


def assemble_output(results, meta):
    outs = [r["out"] for r in results]
    full = np.concatenate(outs, axis=0).astype(np.float32)
    full = full * meta["gamma"] + meta["beta"]
    return full[:meta["N"]]


def kernel(**inputs):
    """Full-input GAT kernel: shards edges by dst across 8 NeuronCores."""
    from concourse import bass_utils
    inputs = {k: np.asarray(v) for k, v in inputs.items()}
    in_maps, meta = prep_inputs(**inputs)
    nc = build_kernel(meta["NB"], meta["NCH"])
    res = bass_utils.run_bass_kernel_spmd(nc, in_maps, core_ids=list(range(meta["ncores"])))
    return assemble_output(res.results, meta).astype(np.float32)
